# revision 12
# baseline (speedup 1.0000x reference)
"""DeepSeek MoE gate (sigmoid routing, grouped top-k) for 8x Trainium2 NeuronCores.

Strategy: data-parallel over tokens (16384 tokens -> 2048 per core), gate
weight + bias replicated. Host-side sharding stores each core's token slice
pre-transposed and supertile-blocked (x [4, 7168, 512] fp32, fully
contiguous 256KB DMA blocks) and the gate weight as W^T [7168, 256], so the
device streams contraction-major tiles and runs zero input transposes.

Logits are computed in [expert, token] orientation with W-side stationary
operands. Each 128-deep contraction chunk uses one of two modes (alternating,
both accumulating into shared PSUM regions psA at scale 2^10 / psB at 2^17):

C4 chunks (even j) - fp16 main + fp8 DoubleRow cross, cheap on PE:
    a' = fp16(2^10 x)            (ACT scale-cast)
    x8 = e4m3(x)                 (DVE cast)
    r8 = e4m3(2^10 x - a')       (DVE scalar_tensor_tensor, one op)
    psA += bh^T @ a'             (fp16, N=512)         bh  = fp16(w)
    psB += s8^T@x8 + b8^T@r8     (one fp8 DoubleRow)   s8  = e4m3((w-bh)*2^17)
                                                       b8  = e4m3(bh*2^7)
T5 chunks (odd j) - zero-cast bf16-view main, cheap on DVE/ACT:
    ah  = high-u16 strided view of the fp32 x tile (= trunc-bf16(x), free)
    r16 = fp16(x - ah)           (GPSIMD sub, the only elementwise op)
    psA += bT^T@ah + bT^T@r16    (bT = fp16(w*2^10), same stationary)
    psB += s8^T@ah               (plain fp8-stationary matmul)

logits = (psA + 2^-7 psB) * 2^-10, folded into the sigmoid's scale after a
PE transpose of the [e, t] logits back to [t, e] (32 fp32 transposes total).
Then: sigmoid+bias, grouped top-4 masking, native top-8, normalize.
Validated against the reference on the full 16k-token input: 9 tokens with
adjacent-rank swaps of near-tied scores, max weight rel err 5.8e-6.
"""

import os
import sys

sys.path.insert(0, "/opt/trn_rl_repo")

import numpy as np

import concourse.bass as bass
import concourse.mybir as mybir
import concourse.tile as tile
from concourse.bass_utils import run_bass_kernel_spmd
from concourse.masks import make_identity

P = 128
H = 7168
E = 256
G = 8  # n_group
GSZ = E // G
TOPK_G = 4
TOPK = 8
N_CORES = 8
T_FULL = 4 * 4096
T_CORE = T_FULL // N_CORES
HC = H // P  # 56 contraction chunks

ST = 512  # supertile: tokens per PSUM accumulation pass
QC = 4   # chunks per DMA quad (8KB partition lines)
NST = T_CORE // ST
NTB = ST // P

F32 = mybir.dt.float32
F16 = mybir.dt.float16
BF16 = mybir.dt.bfloat16
FP8 = mybir.dt.float8e4
U16 = mybir.dt.uint16
U32 = mybir.dt.uint32

A_SHIFT = 1024.0        # 2^10: x-side prescale (C4) / W-side prescale (T5)
S_SHIFT = 2.0 ** 17     # s8 = e4m3(s * 2^17)
B_SHIFT = 2.0 ** 7      # b8 = e4m3(bh * 2^7); r8(2^10) x b8(2^7) = 2^17
CRS_SCALE = 2.0 ** -7   # psB(2^17) -> 2^10 to match psA
SIG_SCALE = 2.0 ** -10  # final descale folded into the sigmoid

ACT_COPY = mybir.ActivationFunctionType.Copy
ACT_SIGMOID = mybir.ActivationFunctionType.Sigmoid
DR = mybir.MatmulPerfMode.DoubleRow


def is_c4(j):
    return j % 3 != 2


def build_moe_gate(tc: tile.TileContext, x_d, w_d, b_d, wout_d, iout_d, t_core,
                   ctx=None):
    nc = tc.nc
    nst = t_core // ST
    nt = t_core // P

    const_pool = ctx.enter_context(tc.tile_pool(name="const", bufs=1))
    wprep_pool = ctx.enter_context(tc.tile_pool(name="wprep", bufs=2))
    xin_pool = ctx.enter_context(tc.tile_pool(name="xin", bufs=4))
    cvt_pool = ctx.enter_context(tc.tile_pool(name="cvt", bufs=6))
    ps_pool = ctx.enter_context(tc.tile_pool(name="ps", bufs=1, space="PSUM"))
    comb_pool = ctx.enter_context(tc.tile_pool(name="comb", bufs=2))
    sc_pool = ctx.enter_context(tc.tile_pool(name="scores", bufs=3))
    sm_pool = ctx.enter_context(tc.tile_pool(name="small", bufs=4))
    out_pool = ctx.enter_context(tc.tile_pool(name="outs", bufs=1))

    identity = const_pool.tile([P, P], F32)
    make_identity(nc, identity)

    bias_rep = const_pool.tile([P, E], F32)
    nc.sync.dma_start(bias_rep, b_d[None, :].to_broadcast([P, E]))

    # ---- W prep (one-time; DMAs ride the ACT HWDGE ring) ----
    bh = const_pool.tile([P, HC, E], F16)    # fp16(w)          (C4 main)
    bT = const_pool.tile([P, HC, E], F16)    # fp16(w * 2^10)   (T5 main)
    sb8 = const_pool.tile([P, HC, 2, E], FP8)  # [s8 | b8]      (cross terms)
    for j in range(HC):
        wsl = wprep_pool.tile([P, E], F32, tag="wsl")
        nc.scalar.dma_start(wsl, w_d[j * P:(j + 1) * P, :])
        nc.vector.tensor_copy(bh[:, j, :], wsl)
        nc.vector.tensor_scalar_mul(bT[:, j, :], wsl, A_SHIFT)
        s32 = wprep_pool.tile([P, E], F32, tag="s32")
        nc.vector.tensor_sub(s32, wsl, bh[:, j, :])
        nc.scalar.activation(sb8[:, j, 0, :], s32, ACT_COPY, scale=S_SHIFT)
        nc.scalar.activation(sb8[:, j, 1, :], bh[:, j, :], ACT_COPY,
                             scale=B_SHIFT)

    wout_sb = out_pool.tile([P, nt, TOPK], F32)
    iout_sb = out_pool.tile([P, nt, TOPK], U32)

    for st in range(nst):
        # [128e x 2eblk x 512t] accumulators: psA at 2^10, psB at 2^17
        psA = ps_pool.tile([P, 2, ST], F32, tag="psA", bufs=2)
        psB = ps_pool.tile([P, 2, ST], F32, tag="psB", bufs=1)
        for q in range(HC // QC):
            # two DMAs per chunk-quad (4KB contiguous partition lines each):
            # big descriptors for bandwidth, fine-grained for pipelining
            xq = xin_pool.tile([P, QC, ST], F32, tag="xin")
            ring2 = nc.sync if st == 0 else nc.scalar
            nc.sync.dma_start(xq[:, 0:2, :], x_d[st, q, :, 0:2, :])
            ring2.dma_start(xq[:, 2:4, :], x_d[st, q, :, 2:4, :])
            for jc in range(QC):
                j = q * QC + jc
                xsl = xq[:, jc, :]
                sflag = (j == 0)
                pflag = (j == HC - 1)
                if is_c4(j):
                    ap = cvt_pool.tile([P, ST], F16, tag="ap")
                    nc.scalar.activation(ap, xsl, ACT_COPY, scale=A_SHIFT)
                    ar8 = cvt_pool.tile([P, 2, ST], FP8, tag="ar8")
                    nc.vector.tensor_copy(ar8[:, 0, :], xsl)
                    nc.vector.scalar_tensor_tensor(
                        ar8[:, 1, :], xsl, A_SHIFT, ap,
                        op0=mybir.AluOpType.mult, op1=mybir.AluOpType.subtract)
                    # alternate DR-cross / fp16-main so LDWEIGHTS hides; at
                    # j==0 run mains first: they only need psA (double-
                    # buffered), giving ACT time to drain the previous
                    # supertile's single-buffered psB
                    for e2 in range(2):
                        if not sflag:
                            nc.tensor.matmul(psB[:, e2, :],
                                             sb8[:, j, :, e2 * P:(e2 + 1) * P],
                                             ar8, start=False, stop=pflag,
                                             perf_mode=DR)
                        nc.tensor.matmul(psA[:, e2, :],
                                         bh[:, j, e2 * P:(e2 + 1) * P], ap,
                                         start=sflag, stop=pflag)
                    if sflag:
                        for e2 in range(2):
                            nc.tensor.matmul(psB[:, e2, :],
                                             sb8[:, j, :, e2 * P:(e2 + 1) * P],
                                             ar8, start=True, stop=pflag,
                                             perf_mode=DR)
                else:
                    ah = xsl.bitcast(U16)[:, 1::2].bitcast(BF16)
                    r16 = cvt_pool.tile([P, ST], F16, tag="r16")
                    nc.gpsimd.tensor_sub(r16, xsl, ah)
                    for e2 in range(2):
                        sl_e = slice(e2 * P, (e2 + 1) * P)
                        nc.tensor.matmul(psA[:, e2, :], bT[:, j, sl_e], ah,
                                         start=sflag, stop=False)
                        nc.tensor.matmul(psA[:, e2, :], bT[:, j, sl_e], r16,
                                         start=False, stop=pflag)
                        nc.tensor.matmul(psB[:, e2, :], sb8[:, j, 0, sl_e],
                                         ah, start=sflag, stop=pflag)

        # logits*2^10 = psA + 2^-7 * psB  (ACT stages psB so the DVE add
        # reads one PSUM + one SBUF operand)
        crs = comb_pool.tile([P, 2, ST], F32, tag="crs")
        nc.scalar.activation(crs, psB, ACT_COPY, scale=CRS_SCALE)
        lgt = comb_pool.tile([P, 2, ST], F32, tag="lgt")
        nc.vector.tensor_add(lgt, psA, crs)

        # per 128-token block: PE-transpose to [t, e], sigmoid (descale), route
        for tb in range(NTB):
            i = st * NTB + tb
            ps_t = ps_pool.tile([P, 2, P], F32, tag="ps_t", bufs=2)
            for e2 in range(2):
                nc.tensor.matmul(ps_t[:, e2, :],
                                 lgt[:, e2, tb * P:(tb + 1) * P], identity,
                                 is_transpose=True)

            scores = sc_pool.tile([P, 2, P], F32, tag="scores")
            nc.scalar.activation(scores, ps_t, ACT_SIGMOID, scale=SIG_SCALE)
            scores = scores.rearrange("p a b -> p (a b)")
            nc.gpsimd.tensor_add(scores, scores, bias_rep)

            scores_g = scores.rearrange("p (g e) -> p g e", g=G)
            gmax = sm_pool.tile([P, G], F32, tag="gmax")
            nc.vector.reduce_max(gmax, scores_g, axis=mybir.AxisListType.X)

            g8 = sm_pool.tile([P, 8], F32, tag="g8")
            nc.vector.max(out=g8, in_=gmax)

            gmask = sm_pool.tile([P, G], F32, tag="gmask")
            nc.vector.tensor_scalar(gmask, gmax, g8[:, TOPK_G - 1:TOPK_G],
                                    None, op0=mybir.AluOpType.is_ge)

            masked = sc_pool.tile([P, E], F32, tag="masked")
            nc.gpsimd.tensor_tensor(
                masked.rearrange("p (g e) -> p g e", g=G), scores_g,
                gmask[:, :, None].to_broadcast([P, G, GSZ]),
                op=mybir.AluOpType.mult)

            m8 = sm_pool.tile([P, 8], F32, tag="m8")
            nc.vector.max(out=m8, in_=masked)
            nc.vector.max_index(iout_sb[:, i, :], m8, masked)

            ssum = sm_pool.tile([P, 1], F32, tag="ssum")
            nc.vector.reduce_sum(ssum, m8, axis=mybir.AxisListType.X)
            nc.vector.tensor_scalar_add(ssum, ssum, 1e-6)
            rcp = sm_pool.tile([P, 1], F32, tag="rcp")
            nc.vector.reciprocal(rcp, ssum)
            nc.vector.tensor_scalar_mul(wout_sb[:, i, :], m8, rcp)

        i0 = st * NTB
        nc.sync.dma_start(wout_d[:, i0:i0 + NTB, :], wout_sb[:, i0:i0 + NTB, :])
        nc.sync.dma_start(iout_d[:, i0:i0 + NTB, :], iout_sb[:, i0:i0 + NTB, :])




def build_bass(t_core=T_CORE):
    from concourse import bacc
    nc = bacc.Bacc("TRN2", target_bir_lowering=False, debug=False,
                   num_devices=N_CORES)
    nst = t_core // ST
    nt = t_core // P
    # x: token slice pre-transposed + supertile-blocked: [nst, H, ST]
    x_d = nc.dram_tensor("x", [nst, HC // QC, P, QC, ST], F32,
                     kind="ExternalInput").ap()
    w_d = nc.dram_tensor("w", [H, E], F32, kind="ExternalInput").ap()
    b_d = nc.dram_tensor("b", [E], F32, kind="ExternalInput").ap()
    wout_d = nc.dram_tensor("wout", [P, nt, TOPK], F32,
                            kind="ExternalOutput").ap()
    iout_d = nc.dram_tensor("iout", [P, nt, TOPK], U32,
                            kind="ExternalOutput").ap()
    from contextlib import ExitStack
    with tile.TileContext(nc) as tc:
        with ExitStack() as ctx:
            build_moe_gate(tc, x_d, w_d, b_d, wout_d, iout_d, t_core, ctx=ctx)
    nc.compile()
    return nc


_NC_CACHE = {}


def _get_nc():
    key = "main"
    if key not in _NC_CACHE:
        _NC_CACHE[key] = build_bass()
    return _NC_CACHE[key]


def _shard_x(xf_slice):
    """[t, H] fp32 -> [t//ST, HC//QC, P, QC, ST]: supertile-blocked x^T with
    chunk-quads interleaved per partition so DMA lines are QC*ST*4 = 8KB."""
    t = xf_slice.shape[0]
    xt = xf_slice.T  # [H, t]
    v = xt.reshape(HC // QC, QC, P, t // ST, ST)
    return np.ascontiguousarray(v.transpose(3, 0, 2, 1, 4))


def kernel(hidden_states, gate_weight, bias, n_group, topk_group, top_k,
           _trace=False):
    assert int(n_group) == G and int(topk_group) == TOPK_G and int(top_k) == TOPK
    x = np.asarray(hidden_states, dtype=np.float32)
    w = np.asarray(gate_weight, dtype=np.float32)
    b = np.ascontiguousarray(np.asarray(bias, dtype=np.float32))
    B, S, _ = x.shape
    xf = x.reshape(-1, H)
    assert xf.shape[0] == T_FULL

    wT = np.ascontiguousarray(w.T)  # [H, E]

    nc = _get_nc()
    in_maps = []
    for c in range(N_CORES):
        in_maps.append({
            "x": _shard_x(xf[c * T_CORE:(c + 1) * T_CORE]),
            "w": wT,
            "b": b,
        })
    try:
        res = run_bass_kernel_spmd(nc, in_maps, core_ids=list(range(N_CORES)),
                                   trace=_trace)
    except ModuleNotFoundError:
        res = run_bass_kernel_spmd(nc, in_maps, core_ids=list(range(N_CORES)),
                                   trace=False)
    weights = np.empty((T_FULL, TOPK), dtype=np.float32)
    indices = np.empty((T_FULL, TOPK), dtype=np.int32)
    for c, r in enumerate(res.results):
        wc = np.transpose(r["wout"], (1, 0, 2)).reshape(T_CORE, TOPK)
        ic = np.transpose(r["iout"], (1, 0, 2)).reshape(T_CORE, TOPK)
        weights[c * T_CORE:(c + 1) * T_CORE] = wc
        indices[c * T_CORE:(c + 1) * T_CORE] = ic.astype(np.int32)
    out_w = weights.reshape(B, S, TOPK)
    out_i = indices.reshape(B, S, TOPK)
    if _trace:
        return (out_w, out_i), res
    return out_w, out_i


# revision 13
# speedup vs baseline: 1.0100x; 1.0100x over previous
"""DeepSeek MoE gate (sigmoid routing, grouped top-k) for 8x Trainium2 NeuronCores.

Strategy: data-parallel over tokens (16384 tokens -> 2048 per core), gate
weight + bias replicated. Host-side sharding stores each core's token slice
pre-transposed and supertile-blocked (x [4, 7168, 512] fp32, fully
contiguous 256KB DMA blocks) and the gate weight as W^T [7168, 256], so the
device streams contraction-major tiles and runs zero input transposes.

Logits are computed in [expert, token] orientation with W-side stationary
operands. Each 128-deep contraction chunk uses one of two modes (alternating,
both accumulating into shared PSUM regions psA at scale 2^10 / psB at 2^17):

C4 chunks (even j) - fp16 main + fp8 DoubleRow cross, cheap on PE:
    a' = fp16(2^10 x)            (ACT scale-cast)
    x8 = e4m3(x)                 (DVE cast)
    r8 = e4m3(2^10 x - a')       (DVE scalar_tensor_tensor, one op)
    psA += bh^T @ a'             (fp16, N=512)         bh  = fp16(w)
    psB += s8^T@x8 + b8^T@r8     (one fp8 DoubleRow)   s8  = e4m3((w-bh)*2^17)
                                                       b8  = e4m3(bh*2^7)
T5 chunks (odd j) - zero-cast bf16-view main, cheap on DVE/ACT:
    ah  = high-u16 strided view of the fp32 x tile (= trunc-bf16(x), free)
    r16 = fp16(x - ah)           (GPSIMD sub, the only elementwise op)
    psA += bT^T@ah + bT^T@r16    (bT = fp16(w*2^10), same stationary)
    psB += s8^T@ah               (plain fp8-stationary matmul)

logits = (psA + 2^-7 psB) * 2^-10, folded into the sigmoid's scale after a
PE transpose of the [e, t] logits back to [t, e] (32 fp32 transposes total).
Then: sigmoid+bias, grouped top-4 masking, native top-8, normalize.
Validated against the reference on the full 16k-token input: 9 tokens with
adjacent-rank swaps of near-tied scores, max weight rel err 5.8e-6.
"""

import os
import sys

sys.path.insert(0, "/opt/trn_rl_repo")

import numpy as np

import concourse.bass as bass
import concourse.mybir as mybir
import concourse.tile as tile
from concourse.bass_utils import run_bass_kernel_spmd
from concourse.masks import make_identity

P = 128
H = 7168
E = 256
G = 8  # n_group
GSZ = E // G
TOPK_G = 4
TOPK = 8
N_CORES = 8
T_FULL = 4 * 4096
T_CORE = T_FULL // N_CORES
HC = H // P  # 56 contraction chunks

ST = 512  # supertile: tokens per PSUM accumulation pass
QC = 4   # chunks per DMA quad (8KB partition lines)
NST = T_CORE // ST
NTB = ST // P

F32 = mybir.dt.float32
F16 = mybir.dt.float16
BF16 = mybir.dt.bfloat16
FP8 = mybir.dt.float8e4
U16 = mybir.dt.uint16
U32 = mybir.dt.uint32

A_SHIFT = 1024.0        # 2^10: x-side prescale (C4) / W-side prescale (T5)
S_SHIFT = 2.0 ** 17     # s8 = e4m3(s * 2^17)
B_SHIFT = 2.0 ** 7      # b8 = e4m3(bh * 2^7); r8(2^10) x b8(2^7) = 2^17
CRS_SCALE = 2.0 ** -7   # psB(2^17) -> 2^10 to match psA
SIG_SCALE = 2.0 ** -10  # final descale folded into the sigmoid

ACT_COPY = mybir.ActivationFunctionType.Copy
ACT_SIGMOID = mybir.ActivationFunctionType.Sigmoid
DR = mybir.MatmulPerfMode.DoubleRow


def is_c4(j):
    return j % 3 != 2


def build_moe_gate(tc: tile.TileContext, x_d, w_d, b_d, wout_d, iout_d, t_core,
                   ctx=None):
    nc = tc.nc
    nst = t_core // ST
    nt = t_core // P

    const_pool = ctx.enter_context(tc.tile_pool(name="const", bufs=1))
    wprep_pool = ctx.enter_context(tc.tile_pool(name="wprep", bufs=2))
    xin_pool = ctx.enter_context(tc.tile_pool(name="xin", bufs=4))
    cvt_pool = ctx.enter_context(tc.tile_pool(name="cvt", bufs=6))
    ps_pool = ctx.enter_context(tc.tile_pool(name="ps", bufs=1, space="PSUM"))
    comb_pool = ctx.enter_context(tc.tile_pool(name="comb", bufs=2))
    sc_pool = ctx.enter_context(tc.tile_pool(name="scores", bufs=3))
    sm_pool = ctx.enter_context(tc.tile_pool(name="small", bufs=4))
    out_pool = ctx.enter_context(tc.tile_pool(name="outs", bufs=1))

    identity = const_pool.tile([P, P], F32)
    make_identity(nc, identity)

    bias_rep = const_pool.tile([P, E], F32)
    nc.sync.dma_start(bias_rep, b_d[None, :].to_broadcast([P, E]))

    # ---- W prep (one-time; DMAs ride the ACT HWDGE ring) ----
    bh = const_pool.tile([P, HC, E], F16)    # fp16(w)          (C4 main)
    bT = const_pool.tile([P, HC, E], F16)    # fp16(w * 2^10)   (T5 main)
    sb8 = const_pool.tile([P, HC, 2, E], FP8)  # [s8 | b8]      (cross terms)
    for j in range(HC):
        wsl = wprep_pool.tile([P, E], F32, tag="wsl")
        nc.scalar.dma_start(wsl, w_d[j * P:(j + 1) * P, :])
        nc.vector.tensor_copy(bh[:, j, :], wsl)
        nc.vector.tensor_scalar_mul(bT[:, j, :], wsl, A_SHIFT)
        s32 = wprep_pool.tile([P, E], F32, tag="s32")
        nc.vector.tensor_sub(s32, wsl, bh[:, j, :])
        nc.scalar.activation(sb8[:, j, 0, :], s32, ACT_COPY, scale=S_SHIFT)
        nc.scalar.activation(sb8[:, j, 1, :], bh[:, j, :], ACT_COPY,
                             scale=B_SHIFT)

    wout_sb = out_pool.tile([P, nt, TOPK], F32)
    iout_sb = out_pool.tile([P, nt, TOPK], U32)

    for st in range(nst):
        # [128e x 2eblk x 512t] accumulators: psA at 2^10, psB at 2^17
        psA = ps_pool.tile([P, 2, ST], F32, tag="psA", bufs=2)
        psB = ps_pool.tile([P, 2, ST], F32, tag="psB", bufs=1)
        for q in range(HC // QC):
            # two DMAs per chunk-quad (4KB contiguous partition lines each):
            # big descriptors for bandwidth, fine-grained for pipelining
            xq = xin_pool.tile([P, QC, ST], F32, tag="xin")
            nc.sync.dma_start(xq[:, 0:2, :], x_d[st, q, :, 0:2, :])
            nc.scalar.dma_start(xq[:, 2:4, :], x_d[st, q, :, 2:4, :])
            for jc in range(QC):
                j = q * QC + jc
                xsl = xq[:, jc, :]
                sflag = (j == 0)
                pflag = (j == HC - 1)
                if is_c4(j):
                    ap = cvt_pool.tile([P, ST], F16, tag="ap")
                    nc.scalar.activation(ap, xsl, ACT_COPY, scale=A_SHIFT)
                    ar8 = cvt_pool.tile([P, 2, ST], FP8, tag="ar8")
                    nc.vector.tensor_copy(ar8[:, 0, :], xsl)
                    nc.vector.scalar_tensor_tensor(
                        ar8[:, 1, :], xsl, A_SHIFT, ap,
                        op0=mybir.AluOpType.mult, op1=mybir.AluOpType.subtract)
                    # alternate DR-cross / fp16-main so LDWEIGHTS hides; at
                    # j==0 run mains first: they only need psA (double-
                    # buffered), giving ACT time to drain the previous
                    # supertile's single-buffered psB
                    for e2 in range(2):
                        if not sflag:
                            nc.tensor.matmul(psB[:, e2, :],
                                             sb8[:, j, :, e2 * P:(e2 + 1) * P],
                                             ar8, start=False, stop=pflag,
                                             perf_mode=DR)
                        nc.tensor.matmul(psA[:, e2, :],
                                         bh[:, j, e2 * P:(e2 + 1) * P], ap,
                                         start=sflag, stop=pflag)
                    if sflag:
                        for e2 in range(2):
                            nc.tensor.matmul(psB[:, e2, :],
                                             sb8[:, j, :, e2 * P:(e2 + 1) * P],
                                             ar8, start=True, stop=pflag,
                                             perf_mode=DR)
                else:
                    ah = xsl.bitcast(U16)[:, 1::2].bitcast(BF16)
                    r16 = cvt_pool.tile([P, ST], F16, tag="r16")
                    nc.gpsimd.tensor_sub(r16, xsl, ah)
                    for e2 in range(2):
                        sl_e = slice(e2 * P, (e2 + 1) * P)
                        nc.tensor.matmul(psA[:, e2, :], bT[:, j, sl_e], ah,
                                         start=sflag, stop=False)
                        nc.tensor.matmul(psA[:, e2, :], bT[:, j, sl_e], r16,
                                         start=False, stop=pflag)
                        nc.tensor.matmul(psB[:, e2, :], sb8[:, j, 0, sl_e],
                                         ah, start=sflag, stop=pflag)

        # logits*2^10 = psA + 2^-7 * psB  (ACT stages psB so the DVE add
        # reads one PSUM + one SBUF operand)
        crs = comb_pool.tile([P, 2, ST], F32, tag="crs")
        nc.scalar.activation(crs, psB, ACT_COPY, scale=CRS_SCALE)
        lgt = comb_pool.tile([P, 2, ST], F32, tag="lgt")
        nc.vector.tensor_add(lgt, psA, crs)

        # per 128-token block: PE-transpose to [t, e], sigmoid (descale), route
        for tb in range(NTB):
            i = st * NTB + tb
            ps_t = ps_pool.tile([P, 2, P], F32, tag="ps_t", bufs=2)
            for e2 in range(2):
                nc.tensor.matmul(ps_t[:, e2, :],
                                 lgt[:, e2, tb * P:(tb + 1) * P], identity,
                                 is_transpose=True)

            scores = sc_pool.tile([P, 2, P], F32, tag="scores")
            nc.scalar.activation(scores, ps_t, ACT_SIGMOID, scale=SIG_SCALE)
            scores = scores.rearrange("p a b -> p (a b)")
            nc.gpsimd.tensor_add(scores, scores, bias_rep)

            scores_g = scores.rearrange("p (g e) -> p g e", g=G)
            gmax = sm_pool.tile([P, G], F32, tag="gmax")
            nc.vector.reduce_max(gmax, scores_g, axis=mybir.AxisListType.X)

            g8 = sm_pool.tile([P, 8], F32, tag="g8")
            nc.vector.max(out=g8, in_=gmax)

            gmask = sm_pool.tile([P, G], F32, tag="gmask")
            nc.vector.tensor_scalar(gmask, gmax, g8[:, TOPK_G - 1:TOPK_G],
                                    None, op0=mybir.AluOpType.is_ge)

            masked = sc_pool.tile([P, E], F32, tag="masked")
            nc.gpsimd.tensor_tensor(
                masked.rearrange("p (g e) -> p g e", g=G), scores_g,
                gmask[:, :, None].to_broadcast([P, G, GSZ]),
                op=mybir.AluOpType.mult)

            m8 = sm_pool.tile([P, 8], F32, tag="m8")
            nc.vector.max(out=m8, in_=masked)
            nc.vector.max_index(iout_sb[:, i, :], m8, masked)

            ssum = sm_pool.tile([P, 1], F32, tag="ssum")
            nc.vector.reduce_sum(ssum, m8, axis=mybir.AxisListType.X)
            nc.vector.tensor_scalar_add(ssum, ssum, 1e-6)
            rcp = sm_pool.tile([P, 1], F32, tag="rcp")
            nc.vector.reciprocal(rcp, ssum)
            nc.vector.tensor_scalar_mul(wout_sb[:, i, :], m8, rcp)

        i0 = st * NTB
        nc.sync.dma_start(wout_d[:, i0:i0 + NTB, :], wout_sb[:, i0:i0 + NTB, :])
        nc.sync.dma_start(iout_d[:, i0:i0 + NTB, :], iout_sb[:, i0:i0 + NTB, :])




def build_bass(t_core=T_CORE):
    from concourse import bacc
    nc = bacc.Bacc("TRN2", target_bir_lowering=False, debug=False,
                   num_devices=N_CORES)
    nst = t_core // ST
    nt = t_core // P
    # x: token slice pre-transposed + supertile-blocked: [nst, H, ST]
    x_d = nc.dram_tensor("x", [nst, HC // QC, P, QC, ST], F32,
                     kind="ExternalInput").ap()
    w_d = nc.dram_tensor("w", [H, E], F32, kind="ExternalInput").ap()
    b_d = nc.dram_tensor("b", [E], F32, kind="ExternalInput").ap()
    wout_d = nc.dram_tensor("wout", [P, nt, TOPK], F32,
                            kind="ExternalOutput").ap()
    iout_d = nc.dram_tensor("iout", [P, nt, TOPK], U32,
                            kind="ExternalOutput").ap()
    from contextlib import ExitStack
    with tile.TileContext(nc) as tc:
        with ExitStack() as ctx:
            build_moe_gate(tc, x_d, w_d, b_d, wout_d, iout_d, t_core, ctx=ctx)
    nc.compile()
    return nc


_NC_CACHE = {}


def _get_nc():
    key = "main"
    if key not in _NC_CACHE:
        _NC_CACHE[key] = build_bass()
    return _NC_CACHE[key]


def _shard_x(xf_slice):
    """[t, H] fp32 -> [t//ST, HC//QC, P, QC, ST]: supertile-blocked x^T with
    chunk-quads interleaved per partition so DMA lines are QC*ST*4 = 8KB."""
    t = xf_slice.shape[0]
    xt = xf_slice.T  # [H, t]
    v = xt.reshape(HC // QC, QC, P, t // ST, ST)
    return np.ascontiguousarray(v.transpose(3, 0, 2, 1, 4))


def kernel(hidden_states, gate_weight, bias, n_group, topk_group, top_k,
           _trace=False):
    assert int(n_group) == G and int(topk_group) == TOPK_G and int(top_k) == TOPK
    x = np.asarray(hidden_states, dtype=np.float32)
    w = np.asarray(gate_weight, dtype=np.float32)
    b = np.ascontiguousarray(np.asarray(bias, dtype=np.float32))
    B, S, _ = x.shape
    xf = x.reshape(-1, H)
    assert xf.shape[0] == T_FULL

    wT = np.ascontiguousarray(w.T)  # [H, E]

    nc = _get_nc()
    in_maps = []
    for c in range(N_CORES):
        in_maps.append({
            "x": _shard_x(xf[c * T_CORE:(c + 1) * T_CORE]),
            "w": wT,
            "b": b,
        })
    try:
        res = run_bass_kernel_spmd(nc, in_maps, core_ids=list(range(N_CORES)),
                                   trace=_trace)
    except ModuleNotFoundError:
        res = run_bass_kernel_spmd(nc, in_maps, core_ids=list(range(N_CORES)),
                                   trace=False)
    weights = np.empty((T_FULL, TOPK), dtype=np.float32)
    indices = np.empty((T_FULL, TOPK), dtype=np.int32)
    for c, r in enumerate(res.results):
        wc = np.transpose(r["wout"], (1, 0, 2)).reshape(T_CORE, TOPK)
        ic = np.transpose(r["iout"], (1, 0, 2)).reshape(T_CORE, TOPK)
        weights[c * T_CORE:(c + 1) * T_CORE] = wc
        indices[c * T_CORE:(c + 1) * T_CORE] = ic.astype(np.int32)
    out_w = weights.reshape(B, S, TOPK)
    out_i = indices.reshape(B, S, TOPK)
    if _trace:
        return (out_w, out_i), res
    return out_w, out_i


# revision 14
# speedup vs baseline: 1.0531x; 1.0428x over previous
"""DeepSeek MoE gate (sigmoid routing, grouped top-k) for 8x Trainium2 NeuronCores.

Strategy: data-parallel over tokens (16384 tokens -> 2048 per core), gate
weight + bias replicated. Host-side sharding stores each core's token slice
pre-transposed and supertile-blocked (x [4, 7168, 512] fp32, fully
contiguous 256KB DMA blocks) and the gate weight as W^T [7168, 256], so the
device streams contraction-major tiles and runs zero input transposes.

Logits are computed in [expert, token] orientation with W-side stationary
operands. Each 128-deep contraction chunk uses one of two modes (alternating,
both accumulating into shared PSUM regions psA at scale 2^10 / psB at 2^17):

C4 chunks (even j) - fp16 main + fp8 DoubleRow cross, cheap on PE:
    a' = fp16(2^10 x)            (ACT scale-cast)
    x8 = e4m3(x)                 (DVE cast)
    r8 = e4m3(2^10 x - a')       (DVE scalar_tensor_tensor, one op)
    psA += bh^T @ a'             (fp16, N=512)         bh  = fp16(w)
    psB += s8^T@x8 + b8^T@r8     (one fp8 DoubleRow)   s8  = e4m3((w-bh)*2^17)
                                                       b8  = e4m3(bh*2^7)
T5 chunks (odd j) - zero-cast bf16-view main, cheap on DVE/ACT:
    ah  = high-u16 strided view of the fp32 x tile (= trunc-bf16(x), free)
    r16 = fp16(x - ah)           (GPSIMD sub, the only elementwise op)
    psA += bT^T@ah + bT^T@r16    (bT = fp16(w*2^10), same stationary)
    psB += s8^T@ah               (plain fp8-stationary matmul)

logits = (psA + 2^-7 psB) * 2^-10, folded into the sigmoid's scale after a
PE transpose of the [e, t] logits back to [t, e] (32 fp32 transposes total).
Then: sigmoid+bias, grouped top-4 masking, native top-8, normalize.
Validated against the reference on the full 16k-token input: 9 tokens with
adjacent-rank swaps of near-tied scores, max weight rel err 5.8e-6.
"""

import os
import sys

sys.path.insert(0, "/opt/trn_rl_repo")

import numpy as np

import concourse.bass as bass
import concourse.mybir as mybir
import concourse.tile as tile
from concourse.bass_utils import run_bass_kernel_spmd
from concourse.masks import make_identity

P = 128
H = 7168
E = 256
G = 8  # n_group
GSZ = E // G
TOPK_G = 4
TOPK = 8
N_CORES = 8
T_FULL = 4 * 4096
T_CORE = T_FULL // N_CORES
HC = H // P  # 56 contraction chunks

ST = 512  # supertile: tokens per PSUM accumulation pass
QC = 4   # chunks per DMA quad (8KB partition lines)
NST = T_CORE // ST
NTB = ST // P

F32 = mybir.dt.float32
F16 = mybir.dt.float16
BF16 = mybir.dt.bfloat16
FP8 = mybir.dt.float8e4
U16 = mybir.dt.uint16
U32 = mybir.dt.uint32

A_SHIFT = 1024.0        # 2^10: x-side prescale (C4) / W-side prescale (T5)
S_SHIFT = 2.0 ** 17     # s8 = e4m3(s * 2^17)
B_SHIFT = 2.0 ** 7      # b8 = e4m3(bh * 2^7); r8(2^10) x b8(2^7) = 2^17
CRS_SCALE = 2.0 ** -7   # psB(2^17) -> 2^10 to match psA
SIG_SCALE = 2.0 ** -10  # final descale folded into the sigmoid

ACT_COPY = mybir.ActivationFunctionType.Copy
ACT_SIGMOID = mybir.ActivationFunctionType.Sigmoid
DR = mybir.MatmulPerfMode.DoubleRow


def is_c4(j):
    return j % 3 != 2


def build_moe_gate(tc: tile.TileContext, x_d, w_d, b_d, wout_d, iout_d, t_core,
                   ctx=None):
    nc = tc.nc
    nst = t_core // ST
    nt = t_core // P

    const_pool = ctx.enter_context(tc.tile_pool(name="const", bufs=1))
    wprep_pool = ctx.enter_context(tc.tile_pool(name="wprep", bufs=2))
    xin_pool = ctx.enter_context(tc.tile_pool(name="xin", bufs=4))
    cvt_pool = ctx.enter_context(tc.tile_pool(name="cvt", bufs=4))
    ps_pool = ctx.enter_context(tc.tile_pool(name="ps", bufs=1, space="PSUM"))
    comb_pool = ctx.enter_context(tc.tile_pool(name="comb", bufs=2))
    sc_pool = ctx.enter_context(tc.tile_pool(name="scores", bufs=3))
    sm_pool = ctx.enter_context(tc.tile_pool(name="small", bufs=4))
    out_pool = ctx.enter_context(tc.tile_pool(name="outs", bufs=1))

    identity = const_pool.tile([P, P], F32)
    make_identity(nc, identity)

    bias_rep = const_pool.tile([P, E], F32)
    nc.sync.dma_start(bias_rep, b_d[None, :].to_broadcast([P, E]))

    # ---- W prep (one-time; DMAs ride the ACT HWDGE ring) ----
    bh = const_pool.tile([P, HC, E], F16)    # fp16(w)          (C4 main)
    bT = const_pool.tile([P, HC, E], F16)    # fp16(w * 2^10)   (T5 main)
    sb8 = const_pool.tile([P, HC, 2, E], FP8)  # [s8 | b8]      (cross terms)
    for j in range(HC):
        wsl = wprep_pool.tile([P, E], F32, tag="wsl")
        nc.scalar.dma_start(wsl, w_d[j * P:(j + 1) * P, :])
        nc.vector.tensor_copy(bh[:, j, :], wsl)
        nc.vector.tensor_scalar_mul(bT[:, j, :], wsl, A_SHIFT)
        s32 = wprep_pool.tile([P, E], F32, tag="s32")
        nc.vector.tensor_sub(s32, wsl, bh[:, j, :])
        nc.scalar.activation(sb8[:, j, 0, :], s32, ACT_COPY, scale=S_SHIFT)
        nc.scalar.activation(sb8[:, j, 1, :], bh[:, j, :], ACT_COPY,
                             scale=B_SHIFT)

    wout_sb = out_pool.tile([P, nt, TOPK], F32)
    iout_sb = out_pool.tile([P, nt, TOPK], U32)

    for st in range(nst):
        # [128e x 2eblk x 512t] accumulators: psA at 2^10, psB at 2^17
        psA = ps_pool.tile([P, 2, ST], F32, tag="psA", bufs=2)
        psB = ps_pool.tile([P, 2, ST], F32, tag="psB", bufs=1)
        for q in range(HC // QC):
            # two DMAs per chunk-quad (4KB contiguous partition lines each):
            # big descriptors for bandwidth, fine-grained for pipelining
            xq = xin_pool.tile([P, QC, ST], F32, tag="xin")
            nc.sync.dma_start(xq[:, 0:2, :], x_d[st, q, :, 0:2, :])
            nc.scalar.dma_start(xq[:, 2:4, :], x_d[st, q, :, 2:4, :])
            for jc in range(QC):
                j = q * QC + jc
                xsl = xq[:, jc, :]
                sflag = (j == 0)
                pflag = (j == HC - 1)
                if is_c4(j):
                    ap = cvt_pool.tile([P, ST], F16, tag="ap")
                    nc.scalar.activation(ap, xsl, ACT_COPY, scale=A_SHIFT)
                    ar8 = cvt_pool.tile([P, 2, ST], FP8, tag="ar8")
                    nc.vector.tensor_copy(ar8[:, 0, :], xsl)
                    nc.vector.scalar_tensor_tensor(
                        ar8[:, 1, :], xsl, A_SHIFT, ap,
                        op0=mybir.AluOpType.mult, op1=mybir.AluOpType.subtract)
                    # alternate DR-cross / fp16-main so LDWEIGHTS hides
                    for e2 in range(2):
                        nc.tensor.matmul(psB[:, e2, :],
                                         sb8[:, j, :, e2 * P:(e2 + 1) * P],
                                         ar8, start=sflag, stop=pflag,
                                         perf_mode=DR)
                        nc.tensor.matmul(psA[:, e2, :],
                                         bh[:, j, e2 * P:(e2 + 1) * P], ap,
                                         start=sflag, stop=pflag)
                else:
                    ah = xsl.bitcast(U16)[:, 1::2].bitcast(BF16)
                    r16 = cvt_pool.tile([P, ST], F16, tag="r16")
                    nc.gpsimd.tensor_sub(r16, xsl, ah)
                    for e2 in range(2):
                        sl_e = slice(e2 * P, (e2 + 1) * P)
                        nc.tensor.matmul(psA[:, e2, :], bT[:, j, sl_e], ah,
                                         start=sflag, stop=False)
                        nc.tensor.matmul(psA[:, e2, :], bT[:, j, sl_e], r16,
                                         start=False, stop=pflag)
                        nc.tensor.matmul(psB[:, e2, :], sb8[:, j, 0, sl_e],
                                         ah, start=sflag, stop=pflag)

        # logits*2^10 = psA + 2^-7 * psB  (ACT stages psB so the DVE add
        # reads one PSUM + one SBUF operand)
        crs = comb_pool.tile([P, 2, ST], F32, tag="crs")
        nc.scalar.activation(crs, psB, ACT_COPY, scale=CRS_SCALE)
        lgt = comb_pool.tile([P, 2, ST], F32, tag="lgt")
        nc.vector.tensor_add(lgt, psA, crs)

        # per 128-token block: PE-transpose to [t, e], sigmoid (descale), route
        for tb in range(NTB):
            i = st * NTB + tb
            ps_t = ps_pool.tile([P, 2, P], F32, tag="ps_t", bufs=2)
            for e2 in range(2):
                nc.tensor.matmul(ps_t[:, e2, :],
                                 lgt[:, e2, tb * P:(tb + 1) * P], identity,
                                 is_transpose=True)

            scores = sc_pool.tile([P, 2, P], F32, tag="scores")
            nc.scalar.activation(scores, ps_t, ACT_SIGMOID, scale=SIG_SCALE)
            scores = scores.rearrange("p a b -> p (a b)")
            nc.gpsimd.tensor_add(scores, scores, bias_rep)

            scores_g = scores.rearrange("p (g e) -> p g e", g=G)
            gmax = sm_pool.tile([P, G], F32, tag="gmax")
            nc.vector.reduce_max(gmax, scores_g, axis=mybir.AxisListType.X)

            g8 = sm_pool.tile([P, 8], F32, tag="g8")
            nc.vector.max(out=g8, in_=gmax)

            gmask = sm_pool.tile([P, G], F32, tag="gmask")
            nc.vector.tensor_scalar(gmask, gmax, g8[:, TOPK_G - 1:TOPK_G],
                                    None, op0=mybir.AluOpType.is_ge)

            masked = sc_pool.tile([P, E], F32, tag="masked")
            nc.gpsimd.tensor_tensor(
                masked.rearrange("p (g e) -> p g e", g=G), scores_g,
                gmask[:, :, None].to_broadcast([P, G, GSZ]),
                op=mybir.AluOpType.mult)

            m8 = sm_pool.tile([P, 8], F32, tag="m8")
            nc.vector.max(out=m8, in_=masked)
            nc.vector.max_index(iout_sb[:, i, :], m8, masked)

            ssum = sm_pool.tile([P, 1], F32, tag="ssum")
            nc.vector.reduce_sum(ssum, m8, axis=mybir.AxisListType.X)
            nc.vector.tensor_scalar_add(ssum, ssum, 1e-6)
            rcp = sm_pool.tile([P, 1], F32, tag="rcp")
            nc.vector.reciprocal(rcp, ssum)
            nc.vector.tensor_scalar_mul(wout_sb[:, i, :], m8, rcp)

    nc.sync.dma_start(wout_d, wout_sb)
    nc.sync.dma_start(iout_d, iout_sb)




def build_bass(t_core=T_CORE):
    from concourse import bacc
    nc = bacc.Bacc("TRN2", target_bir_lowering=False, debug=False,
                   num_devices=N_CORES)
    nst = t_core // ST
    nt = t_core // P
    # x: token slice pre-transposed + supertile-blocked: [nst, H, ST]
    x_d = nc.dram_tensor("x", [nst, HC // QC, P, QC, ST], F32,
                     kind="ExternalInput").ap()
    w_d = nc.dram_tensor("w", [H, E], F32, kind="ExternalInput").ap()
    b_d = nc.dram_tensor("b", [E], F32, kind="ExternalInput").ap()
    wout_d = nc.dram_tensor("wout", [P, nt, TOPK], F32,
                            kind="ExternalOutput").ap()
    iout_d = nc.dram_tensor("iout", [P, nt, TOPK], U32,
                            kind="ExternalOutput").ap()
    from contextlib import ExitStack
    with tile.TileContext(nc) as tc:
        with ExitStack() as ctx:
            build_moe_gate(tc, x_d, w_d, b_d, wout_d, iout_d, t_core, ctx=ctx)
    nc.compile()
    return nc


_NC_CACHE = {}


def _get_nc():
    key = "main"
    if key not in _NC_CACHE:
        _NC_CACHE[key] = build_bass()
    return _NC_CACHE[key]


def _shard_x(xf_slice):
    """[t, H] fp32 -> [t//ST, HC//QC, P, QC, ST]: supertile-blocked x^T with
    chunk-quads interleaved per partition so DMA lines are QC*ST*4 = 8KB."""
    t = xf_slice.shape[0]
    xt = xf_slice.T  # [H, t]
    v = xt.reshape(HC // QC, QC, P, t // ST, ST)
    return np.ascontiguousarray(v.transpose(3, 0, 2, 1, 4))


def kernel(hidden_states, gate_weight, bias, n_group, topk_group, top_k,
           _trace=False):
    assert int(n_group) == G and int(topk_group) == TOPK_G and int(top_k) == TOPK
    x = np.asarray(hidden_states, dtype=np.float32)
    w = np.asarray(gate_weight, dtype=np.float32)
    b = np.ascontiguousarray(np.asarray(bias, dtype=np.float32))
    B, S, _ = x.shape
    xf = x.reshape(-1, H)
    assert xf.shape[0] == T_FULL

    wT = np.ascontiguousarray(w.T)  # [H, E]

    nc = _get_nc()
    in_maps = []
    for c in range(N_CORES):
        in_maps.append({
            "x": _shard_x(xf[c * T_CORE:(c + 1) * T_CORE]),
            "w": wT,
            "b": b,
        })
    try:
        res = run_bass_kernel_spmd(nc, in_maps, core_ids=list(range(N_CORES)),
                                   trace=_trace)
    except ModuleNotFoundError:
        res = run_bass_kernel_spmd(nc, in_maps, core_ids=list(range(N_CORES)),
                                   trace=False)
    weights = np.empty((T_FULL, TOPK), dtype=np.float32)
    indices = np.empty((T_FULL, TOPK), dtype=np.int32)
    for c, r in enumerate(res.results):
        wc = np.transpose(r["wout"], (1, 0, 2)).reshape(T_CORE, TOPK)
        ic = np.transpose(r["iout"], (1, 0, 2)).reshape(T_CORE, TOPK)
        weights[c * T_CORE:(c + 1) * T_CORE] = wc
        indices[c * T_CORE:(c + 1) * T_CORE] = ic.astype(np.int32)
    out_w = weights.reshape(B, S, TOPK)
    out_i = indices.reshape(B, S, TOPK)
    if _trace:
        return (out_w, out_i), res
    return out_w, out_i
